# revision 9
# baseline (speedup 1.0000x reference)
"""Trainium2 Bass kernel for GumbelLatentTyper (eval path).

Contract: kernel(**inputs) takes FULL unsharded inputs
  x [8, 4096, 1024] f32, mask [8, 4096] f32, W [512, 1024] f32,
  codebook [512, 256] f32
and returns the FULL outputs matching reference():
  (out [8, 4096, 256], soft_probs [32768, 512], prob_perplexity scalar,
   gumbel_probs [32768, 512])

Strategy: data-parallel over B across 8 NeuronCores; memory-bound design.
Per core (4096 tokens, 32 tiles of 128):
- logits tile [128 tok, 512 v]: single-pass fp16 matmul (x and W cast to
  fp16; logit abs error ~3e-4 on unit-scale logits).
- softmax on ScalarE (exp with per-partition bias=-max, fused accum sum).
- top-8 values + indices per token via VectorE Max8/MaxIndex; hard one-hot
  as is_equal(logits, max) written densely; out rows via indirect-DMA
  gather of codebook[argmax].
- Host post-pass: tokens whose top-2 logit gap is below a threshold are
  recomputed exactly in fp64 and their hard/out/soft rows fixed, so the
  quantization outputs are exact despite the fp16 matmul. avg_probs /
  perplexity are reduced on host from the returned soft_probs.
"""

import sys

if "/opt/trn_rl_repo" not in sys.path:
    sys.path.insert(0, "/opt/trn_rl_repo")

import numpy as np

B, T, F = 8, 4096, 1024
V, C = 512, 256
NT = T // 128          # 32 token tiles per core
NF = F // 128          # 8 contraction chunks
EPS = 1e-7
GAP_THR = 5e-3         # top-2 logit gap below which the host recomputes

_cached = {}
_run_opts = {}      # test-harness knobs: {"trace": True, ...}
LAST_RESULT = None  # BassKernelResults from the most recent kernel() call


def _build():
    import concourse.bass as bass
    import concourse.tile as tile
    from concourse import bacc, mybir

    fp32 = mybir.dt.float32
    fp16 = mybir.dt.float16
    i32 = mybir.dt.int32
    u32 = mybir.dt.uint32

    nc = bacc.Bacc("TRN2", target_bir_lowering=False, debug=False)

    xh_d = nc.dram_tensor("xh", (NT, 128, NF, 128), fp16, kind="ExternalInput")
    wh_d = nc.dram_tensor("wh", (128, NF, V), fp16, kind="ExternalInput")
    cb_d = nc.dram_tensor("cb", (V, C), fp32, kind="ExternalInput")

    soft_d = nc.dram_tensor("soft", (T, V), fp32, kind="ExternalOutput")
    hard_d = nc.dram_tensor("hard", (T, V), fp32, kind="ExternalOutput")
    outq_d = nc.dram_tensor("outq", (T, C), fp32, kind="ExternalOutput")
    topv_d = nc.dram_tensor("topv", (128, NT * 8), fp32, kind="ExternalOutput")
    topi_d = nc.dram_tensor("topi", (128, NT * 8), u32, kind="ExternalOutput")

    AF = mybir.ActivationFunctionType
    AL = mybir.AluOpType

    with tile.TileContext(nc) as tc:
        with (
            tc.tile_pool(name="const", bufs=1) as cpool,
            tc.tile_pool(name="xin", bufs=4) as xpool,
            tc.tile_pool(name="work", bufs=3) as wpool,
            tc.tile_pool(name="small", bufs=4) as spool,
            tc.tile_pool(name="psum", bufs=4, space="PSUM") as ppool,
        ):
            wh_sb = cpool.tile([128, NF, V], fp16)
            nc.sync.dma_start(wh_sb[:], wh_d[:])

            topv_sb = cpool.tile([128, NT * 8], fp32)
            topi_sb = cpool.tile([128, NT * 8], u32)

            for i in range(NT):
                xh_t = xpool.tile([128, NF, 128], fp16, tag="xh")
                nc.sync.dma_start(xh_t[:], xh_d[i])

                lg_ps = ppool.tile([128, V], fp32, tag="lg")
                for j in range(NF):
                    nc.tensor.matmul(
                        lg_ps[:], xh_t[:, j], wh_sb[:, j],
                        start=(j == 0), stop=(j == NF - 1),
                    )

                logits = wpool.tile([128, V], fp32, tag="logits")
                nc.scalar.copy(logits[:], lg_ps[:])

                # top-8 values + indices (DVE Max8 path)
                tv = topv_sb[:, i * 8:(i + 1) * 8]
                ti = topi_sb[:, i * 8:(i + 1) * 8]
                nc.vector.max(tv, logits[:])
                nc.vector.max_index(ti, tv, logits[:])

                # softmax
                nmx = spool.tile([128, 1], fp32, tag="nmx")
                nc.vector.tensor_scalar_mul(nmx[:], tv[:, 0:1], -1.0)
                exp_t = wpool.tile([128, V], fp32, tag="exp")
                sm = spool.tile([128, 1], fp32, tag="sm")
                nc.scalar.activation(exp_t[:], logits[:], AF.Exp,
                                     bias=nmx[:, 0:1], scale=1.0,
                                     accum_out=sm[:, 0:1])
                r = spool.tile([128, 1], fp32, tag="r")
                nc.vector.reciprocal(r[:], sm[:])
                soft_t = wpool.tile([128, V], fp32, tag="soft")
                nc.scalar.activation(soft_t[:], exp_t[:], AF.Copy,
                                     scale=r[:, 0:1])
                nc.sync.dma_start(soft_d[i * 128:(i + 1) * 128, :], soft_t[:])

                # dense one-hot (1.0 exactly where logits == max)
                hard_t = wpool.tile([128, V], fp32, tag="hard")
                nc.vector.tensor_scalar(hard_t[:], logits[:], tv[:, 0:1], None,
                                        op0=AL.is_equal)
                nc.sync.dma_start(hard_d[i * 128:(i + 1) * 128, :], hard_t[:])

                # out rows: gather codebook[argmax]
                k_i = spool.tile([128, 1], i32, tag="ki")
                nc.vector.tensor_copy(k_i[:], ti[:, 0:1])
                outq_t = wpool.tile([128, C], fp32, tag="outq")
                nc.gpsimd.indirect_dma_start(
                    out=outq_t[:], out_offset=None,
                    in_=cb_d[:],
                    in_offset=bass.IndirectOffsetOnAxis(ap=k_i[:, 0:1], axis=0),
                )
                nc.sync.dma_start(outq_d[i * 128:(i + 1) * 128, :], outq_t[:])

            nc.sync.dma_start(topv_d[:], topv_sb[:])
            nc.sync.dma_start(topi_d[:], topi_sb[:])

    nc.compile()
    return nc


def _get_nc():
    if "nc" not in _cached:
        _cached["nc"] = _build()
    return _cached["nc"]


def kernel(x, mask, W, codebook):
    from concourse.bass_utils import run_bass_kernel_spmd

    global LAST_RESULT
    nc = _get_nc()

    x = np.asarray(x, dtype=np.float32)
    mask = np.asarray(mask, dtype=np.float32)
    W = np.asarray(W, dtype=np.float32)
    codebook = np.asarray(codebook, dtype=np.float32)

    # zero_mask on column 0 folded into weights / codebook row 0
    W0 = W.copy()
    W0[0, :] = 0.0
    cb0 = codebook.copy()
    cb0[0, :] = 0.0

    WT = np.ascontiguousarray(W0.T)                    # [F, V]
    wh = np.ascontiguousarray(
        WT.astype(np.float16).reshape(NF, 128, V).transpose(1, 0, 2))

    xm = x * mask[..., None]                            # fold token mask

    in_maps = []
    for c in range(B):
        xh = np.ascontiguousarray(
            xm[c].astype(np.float16)
            .reshape(NT, 128, NF, 128).transpose(0, 3, 2, 1))
        in_maps.append({"xh": xh, "wh": wh, "cb": cb0})

    LAST_RESULT = run_bass_kernel_spmd(nc, in_maps, list(range(B)),
                                       **_run_opts)
    res = LAST_RESULT.results

    out = np.stack([res[c]["outq"] for c in range(B)])          # [B, T, C]
    soft = np.concatenate([res[c]["soft"] for c in range(B)])   # [B*T, V]
    hard = np.concatenate([res[c]["hard"] for c in range(B)])   # [B*T, V]

    # ---- host post-pass: exact fix of near-tie tokens ----
    # topv [128, NT*8]: token t of core c -> (tile i = t//128, part p = t%128)
    xm2 = xm.reshape(B * T, F)
    fix_rows = []
    for c in range(B):
        tv = res[c]["topv"].reshape(128, NT, 8)
        gap = tv[:, :, 0] - tv[:, :, 1]                  # [128, NT]
        p_idx, i_idx = np.nonzero(gap < GAP_THR)
        fix_rows.extend(c * T + i_idx * 128 + p_idx)
    if len(fix_rows):
        fix_rows = np.asarray(sorted(fix_rows))
        lg64 = xm2[fix_rows].astype(np.float64) @ W0.T.astype(np.float64)
        k_true = np.argmax(lg64, axis=-1)
        hard[fix_rows] = 0.0
        hard[fix_rows, k_true] = 1.0
        out.reshape(B * T, C)[fix_rows] = cb0[k_true]
        # exact (fp32-grade) softmax for the fixed rows
        lg32 = (xm2[fix_rows] @ W0.T).astype(np.float64)
        e = np.exp(lg32 - lg32.max(axis=-1, keepdims=True))
        soft[fix_rows] = (e / e.sum(axis=-1, keepdims=True)).astype(np.float32)

    # ---- avg_probs / perplexity on host (the cross-core all-reduce) ----
    msk = mask.reshape(-1, 1).astype(np.float64)
    sums = (soft.astype(np.float64) * msk).sum(axis=0)
    avg = (sums / msk.sum()).astype(np.float32)
    pp = np.float32(np.exp(-np.sum(avg * np.log(avg + np.float32(EPS)),
                                   dtype=np.float32)))

    return out, soft, pp, hard
